# revision 14
# baseline (speedup 1.0000x reference)
"""ActorHead sparse-attention kernel for 8 TRN2 NeuronCores.

Data-parallel over actors: each core owns 512 of the 4096 actors.
Device layout is fully "transposed" ([m, n] with keys m on partitions,
actor n on free dim) so that:
  - scoresT[m,n] tiles come from one matmul each (lhsT=kT tile, rhs=qT head)
  - softmax reductions over m become PE ones-matvec accumulations in PSUM
  - the ctx matmul consumes e[m,n] directly (lhsT=v tile, rhs=e tile)
Masking: host ships notMaskT (1-mask) bf16; wmT = w - 1000*(1-mask) so
exp(wmT) == 0 exactly at masked keys (fp32 exp underflow).
All heavy tensors bf16; accumulations fp32 in PSUM.

Math (validated vs reference at rel err ~2.5e-3 in bf16):
  f = exp(wmT); t = sum_m f
  e_h = exp(scoresT_h) * f ; s_h = sum_m e_h ; u_h = sum_m e_h*f
  ctx_h = (v_h.T @ e_h) / s_h ; topicT = sum_h WtT_h.T @ ctx_h + bt
  influence = (sum_h u_h/s_h) / (4t)
"""
import math
import numpy as np
import ml_dtypes

import concourse.bass as bass
import concourse.mybir as mybir
import concourse.tile as tile
import concourse.bacc as bacc
from concourse.bass_utils import run_bass_kernel_spmd

BF = ml_dtypes.bfloat16
F32 = mybir.dt.float32
BF16 = mybir.dt.bfloat16

R = 8
N, M, D, H, DK = 4096, 4096, 512, 4, 128
NS = N // R          # 512 actors per core
MT = M // 128        # 32 key tiles
NEGC = -1000.0       # mask offset; exp(w-1000) == 0 in fp32

_cached = {}


def build_nc():
    nc = bacc.Bacc(None, target_bir_lowering=False, debug=False)

    # --- DRAM parameters (per-core shards; names match in_maps keys) ---
    aT = nc.declare_dram_parameter("aT", [D, NS], BF16, isOutput=False)
    bvT = nc.declare_dram_parameter("bvT", [D, M], BF16, isOutput=False)
    nmT = nc.declare_dram_parameter("nmT", [M, NS], BF16, isOutput=False)
    wT = nc.declare_dram_parameter("wT", [M, NS], BF16, isOutput=False)
    wqT = nc.declare_dram_parameter("wqT", [D, D], BF16, isOutput=False)
    wkT = nc.declare_dram_parameter("wkT", [D, D], BF16, isOutput=False)
    wvT = nc.declare_dram_parameter("wvT", [D, D], BF16, isOutput=False)
    bqc = nc.declare_dram_parameter("bqc", [DK, H], F32, isOutput=False)
    bkc = nc.declare_dram_parameter("bkc", [DK, H], F32, isOutput=False)
    bvr = nc.declare_dram_parameter("bvr", [1, D], BF16, isOutput=False)
    WtT = nc.declare_dram_parameter("WtT", [D, DK], BF16, isOutput=False)
    btc = nc.declare_dram_parameter("btc", [DK, 1], F32, isOutput=False)
    ones1 = nc.declare_dram_parameter("ones1", [1, 128], BF16, isOutput=False)
    ones1f = nc.declare_dram_parameter("ones1f", [1, 128], F32, isOutput=False)
    onesP = nc.declare_dram_parameter("onesP", [128, 1], BF16, isOutput=False)
    out = nc.declare_dram_parameter("out", [DK + 1, NS], F32, isOutput=True)

    with tile.TileContext(nc) as tc:
        with (
            tc.tile_pool(name="const", bufs=1) as cpool,
            tc.tile_pool(name="big", bufs=1) as bigpool,
            tc.tile_pool(name="work", bufs=3) as wpool,
            tc.tile_pool(name="pwork", bufs=3) as ppool,
            tc.tile_pool(name="small", bufs=1) as spool,
            tc.tile_pool(name="psA", bufs=2, space="PSUM") as psA,   # build + scores pairs
            tc.tile_pool(name="psB", bufs=1, space="PSUM") as psB,   # ctx
            tc.tile_pool(name="psC", bufs=1, space="PSUM") as psC,   # s
            tc.tile_pool(name="psD", bufs=1, space="PSUM") as psD,   # u
            tc.tile_pool(name="psE", bufs=1, space="PSUM") as psE,   # t
        ):
            # ---- load constants / weights ----
            aT_sb = cpool.tile([128, 4 * NS], BF16, tag="aT")
            nc.sync.dma_start(
                out=aT_sb[:].rearrange("p (t n) -> p t n", n=NS),
                in_=aT.rearrange("(t p) n -> p t n", p=128),
            )
            wq_sb = cpool.tile([128, 4 * D], BF16, tag="wq")
            nc.sync.dma_start(
                out=wq_sb[:].rearrange("p (t n) -> p t n", n=D),
                in_=wqT.rearrange("(t p) n -> p t n", p=128),
            )
            wk_sb = cpool.tile([128, 4 * D], BF16, tag="wk")
            nc.sync.dma_start(
                out=wk_sb[:].rearrange("p (t n) -> p t n", n=D),
                in_=wkT.rearrange("(t p) n -> p t n", p=128),
            )
            wv_sb = cpool.tile([128, 4 * D], BF16, tag="wv")
            nc.sync.dma_start(
                out=wv_sb[:].rearrange("p (t n) -> p t n", n=D),
                in_=wvT.rearrange("(t p) n -> p t n", p=128),
            )
            bq_sb = cpool.tile([128, H], F32, tag="bq")
            nc.sync.dma_start(out=bq_sb[:], in_=bqc[:])
            bk_sb = cpool.tile([128, H], F32, tag="bk")
            nc.sync.dma_start(out=bk_sb[:], in_=bkc[:])
            bv_sb = cpool.tile([1, D], BF16, tag="bv")
            nc.sync.dma_start(out=bv_sb[:], in_=bvr[:])
            Wt_sb = cpool.tile([128, 4 * DK], BF16, tag="Wt")
            nc.sync.dma_start(
                out=Wt_sb[:].rearrange("p (t n) -> p t n", n=DK),
                in_=WtT.rearrange("(t p) n -> p t n", p=128),
            )
            bt_sb = cpool.tile([128, 1], F32, tag="bt")
            nc.sync.dma_start(out=bt_sb[:], in_=btc[:])
            o1_sb = cpool.tile([1, 128], BF16, tag="o1")
            nc.sync.dma_start(out=o1_sb[:], in_=ones1[:])
            o1f_sb = cpool.tile([1, 128], F32, tag="o1f")
            nc.sync.dma_start(out=o1f_sb[:], in_=ones1f[:])
            oP_sb = cpool.tile([128, 1], BF16, tag="oP")
            nc.sync.dma_start(out=oP_sb[:], in_=onesP[:])

            # big streaming inputs
            bvT_sb = bigpool.tile([128, 4 * M], BF16, tag="bigA")
            nc.sync.dma_start(
                out=bvT_sb[:].rearrange("p (t n) -> p t n", n=M),
                in_=bvT.rearrange("(t p) n -> p t n", p=128),
            )

            # ---- qT[h] = wq'_h @ a.T + bq'  -> [DK, NS] per head ----
            qT_sb = cpool.tile([128, H * NS], BF16, tag="qT")
            for h in range(H):
                ps = psA.tile([128, NS], F32, tag="sc")
                for d in range(4):
                    nc.tensor.matmul(
                        ps[:],
                        wq_sb[:, d * D + h * DK:d * D + (h + 1) * DK],
                        aT_sb[:, d * NS:(d + 1) * NS],
                        start=(d == 0), stop=(d == 3),
                    )
                nc.scalar.activation(
                    qT_sb[:, h * NS:(h + 1) * NS], ps[:],
                    mybir.ActivationFunctionType.Identity,
                    bias=bq_sb[:, h:h + 1],
                )

            # ---- kT[h] = wk_h @ bv.T + bk  -> [DK, M] per head ----
            kT_sb = bigpool.tile([128, H * M], BF16, tag="kT")
            for h in range(H):
                for c in range(M // 512):
                    ps = psA.tile([128, 512], F32, tag="sc")
                    for d in range(4):
                        nc.tensor.matmul(
                            ps[:],
                            wk_sb[:, d * D + h * DK:d * D + (h + 1) * DK],
                            bvT_sb[:, d * M + c * 512:d * M + (c + 1) * 512],
                            start=(d == 0), stop=(d == 3),
                        )
                    nc.scalar.activation(
                        kT_sb[:, h * M + c * 512:h * M + (c + 1) * 512], ps[:],
                        mybir.ActivationFunctionType.Identity,
                        bias=bk_sb[:, h:h + 1],
                    )

            # ---- v[mt] = bv[mt-rows] @ wv.T + bv_b -> [128m, D] per m-tile ----
            v_sb = bigpool.tile([128, MT * D], BF16, tag="v")
            for mt in range(MT):
                ps = psA.tile([128, D], F32, tag="sc")
                for d in range(4):
                    nc.tensor.matmul(
                        ps[:],
                        bvT_sb[:, d * M + mt * 128:d * M + (mt + 1) * 128],
                        wv_sb[:, d * D:(d + 1) * D],
                        start=(d == 0), stop=False,
                    )
                nc.tensor.matmul(ps[:], o1_sb[:], bv_sb[:], start=False, stop=True)
                nc.vector.tensor_copy(v_sb[:, mt * D:(mt + 1) * D], ps[:])

            # ---- wmT = w - 1000*(1-mask); f = exp(wmT), chunked [m,n] ----
            # fT reuses bvT's slot (bvT dead once kT/v are built)
            fT_sb = bigpool.tile([128, MT * NS], BF16, tag="bigA")
            G = 4  # m-tiles per chunk
            nm_r = nmT.rearrange("(t p) n -> p t n", p=128)
            w_r = wT.rearrange("(t p) n -> p t n", p=128)
            for g in range(MT // G):
                nmc = wpool.tile([128, G * NS], BF16, tag="nmc")
                nc.sync.dma_start(
                    out=nmc[:].rearrange("p (t n) -> p t n", n=NS),
                    in_=nm_r[:, g * G:(g + 1) * G, :],
                )
                wc = wpool.tile([128, G * NS], BF16, tag="wc")
                nc.sync.dma_start(
                    out=wc[:].rearrange("p (t n) -> p t n", n=NS),
                    in_=w_r[:, g * G:(g + 1) * G, :],
                )
                nc.vector.scalar_tensor_tensor(
                    nmc[:], nmc[:], NEGC, wc[:],
                    mybir.AluOpType.mult, mybir.AluOpType.add,
                )
                nc.scalar.activation(
                    fT_sb[:, g * G * NS:(g + 1) * G * NS], nmc[:],
                    mybir.ActivationFunctionType.Exp,
                )

            # ---- main attention loop: head-major over m-tile pairs ----
            # per-head row vectors live in partition row 0, heads on free dim
            # (engine APs must start at partition 0); s becomes rs in place.
            s_sb = spool.tile([1, H * NS], F32, tag="s")
            u_sb = spool.tile([1, H * NS], F32, tag="u")
            t_sb = spool.tile([1, NS], F32, tag="t")
            ctxn_sb = spool.tile([128, H * NS], BF16, tag="ctxn")

            for h in range(H):
                ctx_ps = psB.tile([128, NS], F32, tag="ctx")
                s_ps = psC.tile([1, NS], F32, tag="s")
                u_ps = psD.tile([1, NS], F32, tag="u")
                if h == 0:
                    t_ps = psE.tile([1, NS], F32, tag="t")
                for pr in range(MT // 2):
                    sc_ps = psA.tile([128, 1024], F32, tag="sc")
                    for half in range(2):
                        mt = 2 * pr + half
                        nc.tensor.matmul(
                            sc_ps[:, half * 512:(half + 1) * 512],
                            kT_sb[:, h * M + mt * 128:h * M + (mt + 1) * 128],
                            qT_sb[:, h * NS:(h + 1) * NS],
                            start=True, stop=True,
                        )
                    e_t = wpool.tile([128, 1024], BF16, tag="e")
                    nc.scalar.activation(
                        e_t[:], sc_ps[:], mybir.ActivationFunctionType.Exp,
                    )
                    fsl = fT_sb[:, 2 * pr * NS:(2 * pr + 2) * NS]
                    nc.vector.tensor_mul(e_t[:], e_t[:], fsl)
                    p_t = ppool.tile([128, 1024], BF16, tag="p")
                    nc.vector.tensor_mul(p_t[:], e_t[:], fsl)
                    for half in range(2):
                        mt = 2 * pr + half
                        st = (pr == 0 and half == 0)
                        sp = (pr == MT // 2 - 1 and half == 1)
                        esl = e_t[:, half * 512:(half + 1) * 512]
                        nc.tensor.matmul(
                            ctx_ps[:],
                            v_sb[:, mt * D + h * DK:mt * D + (h + 1) * DK],
                            esl, start=st, stop=sp,
                        )
                        nc.tensor.matmul(s_ps[:], oP_sb[:], esl, start=st, stop=sp)
                        nc.tensor.matmul(
                            u_ps[:], oP_sb[:],
                            p_t[:, half * 512:(half + 1) * 512],
                            start=st, stop=sp,
                        )
                        if h == 0:
                            nc.tensor.matmul(
                                t_ps[:], oP_sb[:],
                                fT_sb[:, mt * NS:(mt + 1) * NS],
                                start=st, stop=sp,
                            )
                # head tail: s,u -> SBUF; s <- 1/s in place; ctxn = ctx*rs
                ssl = s_sb[:, h * NS:(h + 1) * NS]
                nc.vector.tensor_copy(ssl, s_ps[:])
                nc.vector.tensor_copy(u_sb[:, h * NS:(h + 1) * NS], u_ps[:])
                if h == 0:
                    nc.vector.tensor_copy(t_sb[:], t_ps[:])
                nc.vector.reciprocal(ssl, ssl)
                bc_ps = psA.tile([128, NS], F32, tag="sc")
                nc.tensor.matmul(
                    bc_ps[:], o1f_sb[:], ssl, start=True, stop=True,
                )
                rsb_sb = wpool.tile([128, NS], F32, tag="rsb")
                nc.vector.tensor_copy(rsb_sb[:], bc_ps[:])
                nc.vector.tensor_mul(
                    ctxn_sb[:, h * NS:(h + 1) * NS], ctx_ps[:], rsb_sb[:],
                )

            # ---- topicT = sum_h WtT_h.T @ ctxn_h + bt ----
            tp_ps = psB.tile([128, NS], F32, tag="ctx")
            for h in range(H):
                nc.tensor.matmul(
                    tp_ps[:],
                    Wt_sb[:, h * DK:(h + 1) * DK],
                    ctxn_sb[:, h * NS:(h + 1) * NS],
                    start=(h == 0), stop=(h == 3),
                )
            topic_sb = spool.tile([128, NS], F32, tag="topic")
            nc.scalar.activation(
                topic_sb[:], tp_ps[:],
                mybir.ActivationFunctionType.Identity, bias=bt_sb[:],
            )

            # ---- influence = (sum_h u_h/s_h) / (4t); s_sb already holds 1/s ----
            nc.vector.tensor_mul(u_sb[:], u_sb[:], s_sb[:])
            z0_sb = spool.tile([1, NS], F32, tag="z0")
            z1_sb = spool.tile([1, NS], F32, tag="z1")
            nc.vector.tensor_add(z0_sb[:], u_sb[:, 0:NS], u_sb[:, NS:2 * NS])
            nc.vector.tensor_add(z1_sb[:], u_sb[:, 2 * NS:3 * NS], u_sb[:, 3 * NS:])
            nc.vector.tensor_add(z0_sb[:], z0_sb[:], z1_sb[:])
            nc.vector.reciprocal(t_sb[:], t_sb[:])
            nc.vector.tensor_mul(z0_sb[:], z0_sb[:], t_sb[:])
            inf_sb = spool.tile([1, NS], F32, tag="inf")
            nc.vector.tensor_scalar_mul(inf_sb[:], z0_sb[:], 0.25)

            # ---- write out: rows 0..127 topicT, row 128 influence ----
            nc.sync.dma_start(out=out[0:DK, :], in_=topic_sb[:])
            nc.sync.dma_start(out=out[DK:DK + 1, :], in_=inf_sb[:])

    nc.finalize()
    return nc


def _prep_shared(in_proj_w, in_proj_b, out_w, out_b):
    wq, wk, wv = np.split(np.asarray(in_proj_w, np.float32), 3, axis=0)
    bq, bk, bv_b = np.split(np.asarray(in_proj_b, np.float32), 3)
    sc = 1.0 / math.sqrt(DK)
    Wt = np.asarray(out_w, np.float32).reshape(H, DK, D).mean(axis=0)  # [DK, D]
    bt = np.asarray(out_b, np.float32).reshape(H, DK).mean(axis=0)
    shared = {
        "wqT": np.ascontiguousarray((wq.T * sc).astype(BF)),
        "wkT": np.ascontiguousarray(wk.T.astype(BF)),
        "wvT": np.ascontiguousarray(wv.T.astype(BF)),
        "bqc": np.ascontiguousarray((bq * sc).reshape(H, DK).T.astype(np.float32)),
        "bkc": np.ascontiguousarray(bk.reshape(H, DK).T.astype(np.float32)),
        "bvr": np.ascontiguousarray(bv_b.reshape(1, D).astype(BF)),
        "WtT": np.ascontiguousarray(Wt.T.astype(BF)),
        "btc": np.ascontiguousarray(bt.reshape(DK, 1).astype(np.float32)),
        "ones1": np.ones((1, 128), BF),
        "ones1f": np.ones((1, 128), np.float32),
        "onesP": np.ones((128, 1), BF),
    }
    return shared


def make_in_maps(a_z, bv_z, mask, weight, in_proj_w, in_proj_b, out_w, out_b):
    shared = _prep_shared(in_proj_w, in_proj_b, out_w, out_b)
    a_z = np.asarray(a_z, np.float32)
    bvTfull = np.ascontiguousarray(np.asarray(bv_z, np.float32).T.astype(BF))
    mask = np.asarray(mask)
    weight = np.asarray(weight, np.float32)
    in_maps = []
    for r in range(R):
        sl = slice(r * NS, (r + 1) * NS)
        m = dict(shared)
        m["aT"] = np.ascontiguousarray(a_z[sl].T.astype(BF))
        m["bvT"] = bvTfull
        m["nmT"] = np.ascontiguousarray((~mask[sl]).T.astype(BF))
        m["wT"] = np.ascontiguousarray(weight[sl].T.astype(BF))
        in_maps.append(m)
    return in_maps


def kernel(a_z, bv_z, mask, weight, in_proj_w, in_proj_b, out_w, out_b):
    if "nc" not in _cached:
        _cached["nc"] = build_nc()
    nc = _cached["nc"]
    in_maps = make_in_maps(a_z, bv_z, mask, weight,
                           in_proj_w, in_proj_b, out_w, out_b)
    res = run_bass_kernel_spmd(nc, in_maps, core_ids=list(range(R)))
    outs = [np.asarray(res.results[r]["out"], np.float32) for r in range(R)]
    topic = np.concatenate([o[:DK].T for o in outs], axis=0)
    influence = np.concatenate([o[DK] for o in outs], axis=0)
    return topic, influence


# revision 18
# speedup vs baseline: 1.0597x; 1.0597x over previous
"""ActorHead sparse-attention kernel for 8 TRN2 NeuronCores.

Data-parallel over actors: each core owns 512 of the 4096 actors.
Device layout is fully "transposed" ([m, n] with keys m on partitions,
actor n on free dim) so that:
  - scoresT[m,n] tiles come from one matmul each (lhsT=kT tile, rhs=qT head)
  - softmax reductions over m become PE ones-matvec accumulations in PSUM
  - the ctx matmul consumes e[m,n] directly (lhsT=v tile, rhs=e tile)
Masking: host ships notMaskT (1-mask) bf16; wmT = w - 1000*(1-mask) so
exp(wmT) == 0 exactly at masked keys (fp32 exp underflow).
All heavy tensors bf16; accumulations fp32 in PSUM.

Math (validated vs reference at rel err ~2.5e-3 in bf16):
  f = exp(wmT); t = sum_m f
  e_h = exp(scoresT_h) * f ; s_h = sum_m e_h ; u_h = sum_m e_h*f
  ctx_h = (v_h.T @ e_h) / s_h ; topicT = sum_h WtT_h.T @ ctx_h + bt
  influence = (sum_h u_h/s_h) / (4t)
"""
import math
import numpy as np
import ml_dtypes

import concourse.bass as bass
import concourse.mybir as mybir
import concourse.tile as tile
import concourse.bacc as bacc
from concourse.bass_utils import run_bass_kernel_spmd

BF = ml_dtypes.bfloat16
F32 = mybir.dt.float32
BF16 = mybir.dt.bfloat16

R = 8
N, M, D, H, DK = 4096, 4096, 512, 4, 128
NS = N // R          # 512 actors per core
MT = M // 128        # 32 key tiles
NEGC = -1000.0       # mask offset; exp(w-1000) == 0 in fp32

_cached = {}


def build_nc():
    nc = bacc.Bacc(None, target_bir_lowering=False, debug=False)

    # --- DRAM parameters (per-core shards; names match in_maps keys) ---
    aT = nc.declare_dram_parameter("aT", [D, NS], BF16, isOutput=False)
    bvT = nc.declare_dram_parameter("bvT", [D, NS], BF16, isOutput=False)
    nmT = nc.declare_dram_parameter("nmT", [M, NS], BF16, isOutput=False)
    wT = nc.declare_dram_parameter("wT", [M, NS], BF16, isOutput=False)
    wqT = nc.declare_dram_parameter("wqT", [D, D], BF16, isOutput=False)
    wkT = nc.declare_dram_parameter("wkT", [D, D], BF16, isOutput=False)
    wvT = nc.declare_dram_parameter("wvT", [D, D], BF16, isOutput=False)
    bqc = nc.declare_dram_parameter("bqc", [DK, H], F32, isOutput=False)
    bkc = nc.declare_dram_parameter("bkc", [DK, H], F32, isOutput=False)
    bvr = nc.declare_dram_parameter("bvr", [1, D], BF16, isOutput=False)
    WtT = nc.declare_dram_parameter("WtT", [D, DK], BF16, isOutput=False)
    btc = nc.declare_dram_parameter("btc", [DK, 1], F32, isOutput=False)
    ones1 = nc.declare_dram_parameter("ones1", [1, 128], BF16, isOutput=False)
    ones1f = nc.declare_dram_parameter("ones1f", [1, 128], F32, isOutput=False)
    onesP = nc.declare_dram_parameter("onesP", [128, 1], BF16, isOutput=False)
    out = nc.declare_dram_parameter("out", [DK + 1, NS], F32, isOutput=True)

    with tile.TileContext(nc) as tc:
        with (
            tc.tile_pool(name="const", bufs=1) as cpool,
            tc.tile_pool(name="big", bufs=1) as bigpool,
            tc.tile_pool(name="work", bufs=3) as wpool,
            tc.tile_pool(name="pwork", bufs=3) as ppool,
            tc.tile_pool(name="small", bufs=1) as spool,
            tc.tile_pool(name="psA", bufs=2, space="PSUM") as psA,   # build + scores pairs
            tc.tile_pool(name="psB", bufs=1, space="PSUM") as psB,   # ctx
            tc.tile_pool(name="psC", bufs=1, space="PSUM") as psC,   # s
            tc.tile_pool(name="psD", bufs=1, space="PSUM") as psD,   # u
            tc.tile_pool(name="psE", bufs=1, space="PSUM") as psE,   # t
        ):
            # ---- load constants / weights ----
            aT_sb = cpool.tile([128, 4 * NS], BF16, tag="aT")
            nc.sync.dma_start(
                out=aT_sb[:].rearrange("p (t n) -> p t n", n=NS),
                in_=aT.rearrange("(t p) n -> p t n", p=128),
            )
            wq_sb = cpool.tile([128, 4 * D], BF16, tag="wq")
            nc.sync.dma_start(
                out=wq_sb[:].rearrange("p (t n) -> p t n", n=D),
                in_=wqT.rearrange("(t p) n -> p t n", p=128),
            )
            wk_sb = cpool.tile([128, 4 * D], BF16, tag="wk")
            nc.sync.dma_start(
                out=wk_sb[:].rearrange("p (t n) -> p t n", n=D),
                in_=wkT.rearrange("(t p) n -> p t n", p=128),
            )
            wv_sb = cpool.tile([128, 4 * D], BF16, tag="wv")
            nc.sync.dma_start(
                out=wv_sb[:].rearrange("p (t n) -> p t n", n=D),
                in_=wvT.rearrange("(t p) n -> p t n", p=128),
            )
            bq_sb = cpool.tile([128, H], F32, tag="bq")
            nc.sync.dma_start(out=bq_sb[:], in_=bqc[:])
            bk_sb = cpool.tile([128, H], F32, tag="bk")
            nc.sync.dma_start(out=bk_sb[:], in_=bkc[:])
            bv_sb = cpool.tile([1, D], BF16, tag="bv")
            nc.sync.dma_start(out=bv_sb[:], in_=bvr[:])
            Wt_sb = cpool.tile([128, 4 * DK], BF16, tag="Wt")
            nc.sync.dma_start(
                out=Wt_sb[:].rearrange("p (t n) -> p t n", n=DK),
                in_=WtT.rearrange("(t p) n -> p t n", p=128),
            )
            bt_sb = cpool.tile([128, 1], F32, tag="bt")
            nc.sync.dma_start(out=bt_sb[:], in_=btc[:])
            o1_sb = cpool.tile([1, 128], BF16, tag="o1")
            nc.sync.dma_start(out=o1_sb[:], in_=ones1[:])
            o1f_sb = cpool.tile([1, 128], F32, tag="o1f")
            nc.sync.dma_start(out=o1f_sb[:], in_=ones1f[:])
            oP_sb = cpool.tile([128, 1], BF16, tag="oP")
            nc.sync.dma_start(out=oP_sb[:], in_=onesP[:])

            # this core's 512-row shard of bv (transposed)
            bvT_sb = cpool.tile([128, 4 * NS], BF16, tag="bvT")
            nc.sync.dma_start(
                out=bvT_sb[:].rearrange("p (t n) -> p t n", n=NS),
                in_=bvT.rearrange("(t p) n -> p t n", p=128),
            )

            # ---- qT[h] = wq'_h @ a.T + bq'  -> [DK, NS] per head ----
            qT_sb = cpool.tile([128, H * NS], BF16, tag="qT")
            for h in range(H):
                ps = psA.tile([128, NS], F32, tag="sc")
                for d in range(4):
                    nc.tensor.matmul(
                        ps[:],
                        wq_sb[:, d * D + h * DK:d * D + (h + 1) * DK],
                        aT_sb[:, d * NS:(d + 1) * NS],
                        start=(d == 0), stop=(d == 3),
                    )
                nc.scalar.activation(
                    qT_sb[:, h * NS:(h + 1) * NS], ps[:],
                    mybir.ActivationFunctionType.Identity,
                    bias=bq_sb[:, h:h + 1],
                )

            # ---- k/v: build only this core's 512-key shard, then AllGather ----
            # kT-shard[h] = wk_h @ bv_shard.T + bk -> [DK, NS] per head
            kTs_sb = cpool.tile([128, H * NS], BF16, tag="kTs")
            for h in range(H):
                ps = psA.tile([128, NS], F32, tag="sc")
                for d in range(4):
                    nc.tensor.matmul(
                        ps[:],
                        wk_sb[:, d * D + h * DK:d * D + (h + 1) * DK],
                        bvT_sb[:, d * NS:(d + 1) * NS],
                        start=(d == 0), stop=(d == 3),
                    )
                nc.scalar.activation(
                    kTs_sb[:, h * NS:(h + 1) * NS], ps[:],
                    mybir.ActivationFunctionType.Identity,
                    bias=bk_sb[:, h:h + 1],
                )
            # v-shard[j] = bv_shard[j-rows] @ wv.T + bv_b -> [128m, D], j=0..3
            vs_sb = cpool.tile([128, 4 * D], BF16, tag="vs")
            for j in range(4):
                ps = psA.tile([128, D], F32, tag="sc")
                for d in range(4):
                    nc.tensor.matmul(
                        ps[:],
                        bvT_sb[:, d * NS + j * 128:d * NS + (j + 1) * 128],
                        wv_sb[:, d * D:(d + 1) * D],
                        start=(d == 0), stop=False,
                    )
                nc.tensor.matmul(ps[:], o1_sb[:], bv_sb[:], start=False, stop=True)
                nc.vector.tensor_copy(vs_sb[:, j * D:(j + 1) * D], ps[:])

            # AllGather the 1MB shard pair -> full kT [DK, M] x4 heads + v [M, D]
            with tc.tile_pool(name="dram", bufs=1, space="DRAM") as dpool:
                ag_in = dpool.tile([8 * 128, 512], BF16, tag="agin")
                ag_out = dpool.tile([R * 8 * 128, 512], BF16, tag="agout")
                nc.sync.dma_start(
                    out=ag_in[:].rearrange("(t p) n -> p t n", p=128)[:, 0:4, :],
                    in_=kTs_sb[:].rearrange("p (t n) -> p t n", n=NS),
                )
                nc.sync.dma_start(
                    out=ag_in[:].rearrange("(t p) n -> p t n", p=128)[:, 4:8, :],
                    in_=vs_sb[:].rearrange("p (t n) -> p t n", n=D),
                )
                nc.gpsimd.collective_compute(
                    "AllGather",
                    mybir.AluOpType.bypass,
                    replica_groups=[list(range(R))],
                    ins=[ag_in.opt()],
                    outs=[ag_out.opt()],
                )
                kT_sb = bigpool.tile([128, H * M], BF16, tag="kT")
                v_sb = bigpool.tile([128, MT * D], BF16, tag="v")
                ag_r = ag_out.rearrange("(r t p) n -> r t p n", r=R, p=128)
                for r in range(R):
                    for h in range(H):
                        nc.sync.dma_start(
                            out=kT_sb[:, h * M + r * NS:h * M + (r + 1) * NS],
                            in_=ag_r[r, h],
                        )
                    for j in range(4):
                        mt = r * 4 + j
                        nc.sync.dma_start(
                            out=v_sb[:, mt * D:(mt + 1) * D],
                            in_=ag_r[r, 4 + j],
                        )

            # ---- wmT = w - 1000*(1-mask); f = exp(wmT), chunked [m,n] ----
            # fT reuses bvT's slot (bvT dead once kT/v are built)
            fT_sb = bigpool.tile([128, MT * NS], BF16, tag="bigA")
            G = 4  # m-tiles per chunk
            nm_r = nmT.rearrange("(t p) n -> p t n", p=128)
            w_r = wT.rearrange("(t p) n -> p t n", p=128)
            for g in range(MT // G):
                nmc = wpool.tile([128, G * NS], BF16, tag="nmc")
                nc.sync.dma_start(
                    out=nmc[:].rearrange("p (t n) -> p t n", n=NS),
                    in_=nm_r[:, g * G:(g + 1) * G, :],
                )
                wc = wpool.tile([128, G * NS], BF16, tag="wc")
                nc.sync.dma_start(
                    out=wc[:].rearrange("p (t n) -> p t n", n=NS),
                    in_=w_r[:, g * G:(g + 1) * G, :],
                )
                nc.vector.scalar_tensor_tensor(
                    nmc[:], nmc[:], NEGC, wc[:],
                    mybir.AluOpType.mult, mybir.AluOpType.add,
                )
                nc.scalar.activation(
                    fT_sb[:, g * G * NS:(g + 1) * G * NS], nmc[:],
                    mybir.ActivationFunctionType.Exp,
                )

            # ---- main attention loop: head-major over m-tile pairs ----
            # per-head row vectors live in partition row 0, heads on free dim
            # (engine APs must start at partition 0); s becomes rs in place.
            s_sb = spool.tile([1, H * NS], F32, tag="s")
            u_sb = spool.tile([1, H * NS], F32, tag="u")
            t_sb = spool.tile([1, NS], F32, tag="t")
            ctxn_sb = spool.tile([128, H * NS], BF16, tag="ctxn")

            for h in range(H):
                ctx_ps = psB.tile([128, NS], F32, tag="ctx")
                s_ps = psC.tile([1, NS], F32, tag="s")
                u_ps = psD.tile([1, NS], F32, tag="u")
                if h == 0:
                    t_ps = psE.tile([1, NS], F32, tag="t")
                for pr in range(MT // 2):
                    sc_ps = psA.tile([128, 1024], F32, tag="sc")
                    for half in range(2):
                        mt = 2 * pr + half
                        nc.tensor.matmul(
                            sc_ps[:, half * 512:(half + 1) * 512],
                            kT_sb[:, h * M + mt * 128:h * M + (mt + 1) * 128],
                            qT_sb[:, h * NS:(h + 1) * NS],
                            start=True, stop=True,
                        )
                    e_t = wpool.tile([128, 1024], BF16, tag="e")
                    nc.scalar.activation(
                        e_t[:], sc_ps[:], mybir.ActivationFunctionType.Exp,
                    )
                    fsl = fT_sb[:, 2 * pr * NS:(2 * pr + 2) * NS]
                    nc.vector.tensor_mul(e_t[:], e_t[:], fsl)
                    p_t = ppool.tile([128, 1024], BF16, tag="p")
                    nc.vector.tensor_mul(p_t[:], e_t[:], fsl)
                    for half in range(2):
                        mt = 2 * pr + half
                        st = (pr == 0 and half == 0)
                        sp = (pr == MT // 2 - 1 and half == 1)
                        esl = e_t[:, half * 512:(half + 1) * 512]
                        nc.tensor.matmul(
                            ctx_ps[:],
                            v_sb[:, mt * D + h * DK:mt * D + (h + 1) * DK],
                            esl, start=st, stop=sp,
                        )
                        nc.tensor.matmul(s_ps[:], oP_sb[:], esl, start=st, stop=sp)
                        nc.tensor.matmul(
                            u_ps[:], oP_sb[:],
                            p_t[:, half * 512:(half + 1) * 512],
                            start=st, stop=sp,
                        )
                        if h == 0:
                            nc.tensor.matmul(
                                t_ps[:], oP_sb[:],
                                fT_sb[:, mt * NS:(mt + 1) * NS],
                                start=st, stop=sp,
                            )
                # head tail: s,u -> SBUF; s <- 1/s in place; ctxn = ctx*rs
                ssl = s_sb[:, h * NS:(h + 1) * NS]
                nc.vector.tensor_copy(ssl, s_ps[:])
                nc.vector.tensor_copy(u_sb[:, h * NS:(h + 1) * NS], u_ps[:])
                if h == 0:
                    nc.vector.tensor_copy(t_sb[:], t_ps[:])
                nc.vector.reciprocal(ssl, ssl)
                bc_ps = psA.tile([128, NS], F32, tag="sc")
                nc.tensor.matmul(
                    bc_ps[:], o1f_sb[:], ssl, start=True, stop=True,
                )
                rsb_sb = wpool.tile([128, NS], F32, tag="rsb")
                nc.vector.tensor_copy(rsb_sb[:], bc_ps[:])
                nc.vector.tensor_mul(
                    ctxn_sb[:, h * NS:(h + 1) * NS], ctx_ps[:], rsb_sb[:],
                )

            # ---- topicT = sum_h WtT_h.T @ ctxn_h + bt ----
            tp_ps = psB.tile([128, NS], F32, tag="ctx")
            for h in range(H):
                nc.tensor.matmul(
                    tp_ps[:],
                    Wt_sb[:, h * DK:(h + 1) * DK],
                    ctxn_sb[:, h * NS:(h + 1) * NS],
                    start=(h == 0), stop=(h == 3),
                )
            topic_sb = spool.tile([128, NS], F32, tag="topic")
            nc.scalar.activation(
                topic_sb[:], tp_ps[:],
                mybir.ActivationFunctionType.Identity, bias=bt_sb[:],
            )

            # ---- influence = (sum_h u_h/s_h) / (4t); s_sb already holds 1/s ----
            nc.vector.tensor_mul(u_sb[:], u_sb[:], s_sb[:])
            z0_sb = spool.tile([1, NS], F32, tag="z0")
            z1_sb = spool.tile([1, NS], F32, tag="z1")
            nc.vector.tensor_add(z0_sb[:], u_sb[:, 0:NS], u_sb[:, NS:2 * NS])
            nc.vector.tensor_add(z1_sb[:], u_sb[:, 2 * NS:3 * NS], u_sb[:, 3 * NS:])
            nc.vector.tensor_add(z0_sb[:], z0_sb[:], z1_sb[:])
            nc.vector.reciprocal(t_sb[:], t_sb[:])
            nc.vector.tensor_mul(z0_sb[:], z0_sb[:], t_sb[:])
            inf_sb = spool.tile([1, NS], F32, tag="inf")
            nc.vector.tensor_scalar_mul(inf_sb[:], z0_sb[:], 0.25)

            # ---- write out: rows 0..127 topicT, row 128 influence ----
            nc.sync.dma_start(out=out[0:DK, :], in_=topic_sb[:])
            nc.sync.dma_start(out=out[DK:DK + 1, :], in_=inf_sb[:])

    nc.finalize()
    return nc


def _prep_shared(in_proj_w, in_proj_b, out_w, out_b):
    wq, wk, wv = np.split(np.asarray(in_proj_w, np.float32), 3, axis=0)
    bq, bk, bv_b = np.split(np.asarray(in_proj_b, np.float32), 3)
    sc = 1.0 / math.sqrt(DK)
    Wt = np.asarray(out_w, np.float32).reshape(H, DK, D).mean(axis=0)  # [DK, D]
    bt = np.asarray(out_b, np.float32).reshape(H, DK).mean(axis=0)
    shared = {
        "wqT": np.ascontiguousarray((wq.T * sc).astype(BF)),
        "wkT": np.ascontiguousarray(wk.T.astype(BF)),
        "wvT": np.ascontiguousarray(wv.T.astype(BF)),
        "bqc": np.ascontiguousarray((bq * sc).reshape(H, DK).T.astype(np.float32)),
        "bkc": np.ascontiguousarray(bk.reshape(H, DK).T.astype(np.float32)),
        "bvr": np.ascontiguousarray(bv_b.reshape(1, D).astype(BF)),
        "WtT": np.ascontiguousarray(Wt.T.astype(BF)),
        "btc": np.ascontiguousarray(bt.reshape(DK, 1).astype(np.float32)),
        "ones1": np.ones((1, 128), BF),
        "ones1f": np.ones((1, 128), np.float32),
        "onesP": np.ones((128, 1), BF),
    }
    return shared


def make_in_maps(a_z, bv_z, mask, weight, in_proj_w, in_proj_b, out_w, out_b):
    shared = _prep_shared(in_proj_w, in_proj_b, out_w, out_b)
    a_z = np.asarray(a_z, np.float32)
    bv_z = np.asarray(bv_z, np.float32)
    mask = np.asarray(mask)
    weight = np.asarray(weight, np.float32)
    in_maps = []
    for r in range(R):
        sl = slice(r * NS, (r + 1) * NS)
        m = dict(shared)
        m["aT"] = np.ascontiguousarray(a_z[sl].T.astype(BF))
        m["bvT"] = np.ascontiguousarray(bv_z[sl].T.astype(BF))
        m["nmT"] = np.ascontiguousarray((~mask[sl]).T.astype(BF))
        m["wT"] = np.ascontiguousarray(weight[sl].T.astype(BF))
        in_maps.append(m)
    return in_maps


def kernel(a_z, bv_z, mask, weight, in_proj_w, in_proj_b, out_w, out_b):
    if "nc" not in _cached:
        _cached["nc"] = build_nc()
    nc = _cached["nc"]
    in_maps = make_in_maps(a_z, bv_z, mask, weight,
                           in_proj_w, in_proj_b, out_w, out_b)
    res = run_bass_kernel_spmd(nc, in_maps, core_ids=list(range(R)))
    outs = [np.asarray(res.results[r]["out"], np.float32) for r in range(R)]
    topic = np.concatenate([o[:DK].T for o in outs], axis=0)
    influence = np.concatenate([o[DK] for o in outs], axis=0)
    return topic, influence
